# revision 13
# baseline (speedup 1.0000x reference)
"""Trainium2 Bass kernel v6: per-image routed data augmentation (moe_routing).

For each image i, apply transform sample[i]:
  0: identity  1: fliplr  2: flipud  3: brightness(clip(1.5x,0,1))
  4: contrast(clip(1.5(x-mean)+mean,0,1))  5: solarize(x<0.5 ? x : 1-x)

Learned from HW profiles of earlier versions:
  v3 (264us): predicated-false DMAs still read their full source, and the
      flipud DRAM round-trip tripled read traffic (57.8MB read vs 19.3 needed).
  v4 (282us): SWDGE (gpsimd) casting DMAs serialize on Q7 descriptor gen.
  v5 (320us): PE matmul flipud-selection costs 256 matmuls + 256 ldweights
      (214us at mid p-state) and gpsimd tensor ops are ~2us each.

v6 keeps the lean single-pass ALU pipeline of v5 but does flipud at load
time: the channel-aligned P=96 layout (partition 32c+pc holds rows
[7pc,7pc+7) of channel c) makes the flipped image expressible as ONE
strided DMA from x ([3,32,7,224] with negative h strides), so each image
issues two mutually-exclusive predicated loads (normal / flipped) and no
other flipud machinery exists.  Phantom reads of the skipped variant cost
+19.3MB but remove all PE/PSUM work.

Per image (32 per core, pure data parallel on 8 cores):
    DMA   2 mutually-exclusive predicated loads (SP queue)
    ACT   g = Prelu_al(s*v + b), one instr, accum_out -> row sums rs
    Pool  partition_all_reduce rs -> S (broadcast column)
    DVE   D = fb*S + Dstat (tiny)
    DVE   u3 = Bp*v_wrev + g
    DVE   t = c*u3 + D ; out = clip(t, 0, 1)
    DMA   static batched store, 4 images per DMA (ACT queue)

Transform coefficient table (per-image columns):
    t=0 identity:   s=-1, al=1,  b=0,   Bp=0,  c=-1, D=0
    t=1 fliplr:     s=-1, al=0,  b=0,   Bp=-1, c=-1, D=0
    t=2 flipud:     s=-1, al=1,  b=0,   Bp=0,  c=-1, D=0   (flipped load)
    t=3 brightness: s=1.5,al=1,  b=0,   Bp=0,  c=+1, D=0      (clip does it)
    t=4 contrast:   s=1.5,al=1,  b=0,   Bp=0,  c=+1, D=-S_g/(3*PIX)
    t=5 solarize:   s=-1, al=-1, b=1/2, Bp=0,  c=-1, D=1/2
"""

import numpy as np

import concourse.bass as bass
import concourse.bacc as bacc
import concourse.mybir as mybir
import concourse.bass_isa as bass_isa
from concourse.tile import TileContext
from concourse.bass_utils import run_bass_kernel_spmd

N_CORES = 8
B = 256
B_LOC = B // N_CORES          # 32 images per core
C, H, W = 3, 224, 224
PIX = C * H * W               # 150528
P = 96                        # partitions; 32 per channel, 7 H-rows each
RPP = 7                       # H-rows per partition
FREE = RPP * W                # 1568
GROUP = 4                     # images per store DMA

f32 = mybir.dt.float32
i32 = mybir.dt.int32
Alu = mybir.AluOpType
Act = mybir.ActivationFunctionType

_CACHE = {}
# test harness hook: when _TRACE["on"], kernel() captures an NTFF profile
_TRACE = {"on": False, "res": None}


def _build_nc(n_flip: int):
    nc = bacc.Bacc()
    x = nc.declare_dram_parameter("x", [B_LOC, C, H, W], f32, isOutput=False)
    samp = nc.declare_dram_parameter("sample", [B_LOC], i32, isOutput=False)
    out = nc.declare_dram_parameter("out", [B_LOC, C, H, W], f32, isOutput=True)

    with TileContext(nc) as tc:
        with (
            tc.tile_pool(name="coef", bufs=1) as coef_pool,
            tc.tile_pool(name="data", bufs=6) as data_pool,
            tc.tile_pool(name="work", bufs=3) as work_pool,
            tc.tile_pool(name="outp", bufs=2) as out_pool,
            tc.tile_pool(name="stat", bufs=4) as stat_pool,
        ):
            # ---------- routing coefficients from sample ----------
            s_i = coef_pool.tile([1, B_LOC], i32)
            nc.sync.dma_start(s_i, samp[:].unsqueeze(0))
            s_f = coef_pool.tile([1, B_LOC], f32)
            nc.vector.tensor_copy(s_f, s_i)

            m = {}
            for k in (1, 3, 4, 5):
                mk = coef_pool.tile([1, B_LOC], f32, tag=f"mask{k}")
                nc.vector.tensor_scalar(mk, s_f, float(k), None, Alu.is_equal)
                m[k] = mk
            m34 = coef_pool.tile([1, B_LOC], f32)
            nc.vector.tensor_tensor(m34, m[3], m[4], Alu.add)

            rows = {}
            sc_row = coef_pool.tile([1, B_LOC], f32)   # s = -1 + 2.5*m34
            nc.vector.tensor_scalar(sc_row, m34, 2.5, -1.0, Alu.mult, Alu.add)
            rows["sc"] = sc_row
            t1 = coef_pool.tile([1, B_LOC], f32)       # al = 1 - m1 - 2*m5
            nc.vector.scalar_tensor_tensor(t1, m[5], 2.0, m[1], Alu.mult, Alu.add)
            al_row = coef_pool.tile([1, B_LOC], f32)
            nc.vector.tensor_scalar(al_row, t1, -1.0, 1.0, Alu.mult, Alu.add)
            rows["al"] = al_row
            bb_row = coef_pool.tile([1, B_LOC], f32)   # b = Dstat = 0.5*m5
            nc.vector.tensor_scalar(bb_row, m[5], 0.5, None, Alu.mult)
            rows["bb"] = bb_row
            bp_row = coef_pool.tile([1, B_LOC], f32)   # Bp = -m1
            nc.vector.tensor_scalar(bp_row, m[1], -1.0, None, Alu.mult)
            rows["bp"] = bp_row
            c_row = coef_pool.tile([1, B_LOC], f32)    # c = -1 + 2*m34
            nc.vector.tensor_scalar(c_row, m34, 2.0, -1.0, Alu.mult, Alu.add)
            rows["c"] = c_row
            fb_row = coef_pool.tile([1, B_LOC], f32)   # fb = -m4/(3*PIX)
            nc.vector.tensor_scalar(fb_row, m[4], -1.0 / (3.0 * PIX), None,
                                    Alu.mult)
            rows["fb"] = fb_row

            bc = {}
            for name, row in rows.items():
                t = coef_pool.tile([P, B_LOC], f32, tag=f"bc_{name}")
                nc.gpsimd.partition_broadcast(t, row)
                bc[name] = t

            # int flags for the predicated loads
            is_ud = coef_pool.tile([1, B_LOC], i32)
            nc.vector.tensor_scalar(is_ud, s_i, 2, None, Alu.is_equal)
            not_ud = coef_pool.tile([1, B_LOC], i32)
            nc.vector.tensor_scalar(not_ud, s_i, 2, None, Alu.not_equal)

            # ---------- main loop, one 4-image group at a time ----------
            # The gpsimd cross-partition reduce is batched per group: each
            # image's ACT accum lands in one column of rs4g, a single
            # partition_all_reduce handles all four, and the tail ops
            # (D column, final affine + clamp) run after it.
            kf4 = -((-n_flip) // GROUP) * GROUP   # first fully-plain group
            for i0 in range(0, B_LOC, GROUP):
                og = out_pool.tile([P, GROUP * FREE], f32, tag="og")
                rs4g = stat_pool.tile([P, GROUP], f32, tag="rs4g")
                u3s = []
                tg = None
                if i0 >= kf4:
                    # fully-plain group: one batched load for all 4 images
                    tg = data_pool.tile([P, GROUP * FREE], f32, tag="tg",
                                        bufs=2)
                    gsrc = x[i0:i0 + GROUP].rearrange(
                        "b c h w -> b (c h w)").rearrange(
                        "b (p f) -> p b f", p=P)
                    nc.sync.dma_start(
                        tg.rearrange("p (b f) -> p b f", b=GROUP), gsrc)
                for k in range(GROUP):
                    i = i0 + k
                    # host-side sort puts all flipud images in the first
                    # n_flip slots; only those need the expensive dual
                    # predicated loads (the flipped AP walks 672 DGE
                    # descriptors whether skipped or not).
                    if tg is not None:
                        T = tg[:, k * FREE:(k + 1) * FREE]
                    else:
                        T = data_pool.tile([P, FREE], f32, tag="T", bufs=4)
                        src_n = x[i].flatten().rearrange("(p f) -> p f", p=P)
                        if i < n_flip:
                            cond_n = nc.values_load(
                                not_ud[0:1, i:i + 1],
                                engines=(mybir.EngineType.SP,),
                                min_val=0, max_val=1,
                                skip_runtime_bounds_check=True)
                            cond_u = nc.values_load(
                                is_ud[0:1, i:i + 1],
                                engines=(mybir.EngineType.SP,),
                                min_val=0, max_val=1,
                                skip_runtime_bounds_check=True)
                            nc.sync.dma_start(T, src_n, cond=cond_n,
                                              cond_hint=True)
                            src_u = x[i, :, ::-1, :].rearrange(
                                "c (p r) w -> c p r w", p=32)
                            nc.sync.dma_start(T, src_u, cond=cond_u,
                                              cond_hint=False)
                        else:
                            nc.sync.dma_start(T, src_n)

                    T3 = T.rearrange("p (r w) -> p r w", r=RPP)

                    # ACT1: g = Prelu_al(s*v + b), accum -> per-partition sums
                    g = work_pool.tile([P, FREE], f32, tag="g")
                    nc.scalar.activation(
                        g, T, Act.Prelu,
                        bias=bc["bb"][:, i:i + 1],
                        scale=bc["sc"][:, i:i + 1],
                        alpha=bc["al"][:, i:i + 1],
                        accum_out=rs4g[:, k:k + 1],
                    )

                    # u3 = Bp * v_wrev + g
                    u3 = work_pool.tile([P, FREE], f32, tag="u3", bufs=6)
                    nc.vector.scalar_tensor_tensor(
                        u3.rearrange("p (r w) -> p r w", r=RPP),
                        T3[:, :, ::-1], bc["bp"][:, i:i + 1],
                        g.rearrange("p (r w) -> p r w", r=RPP),
                        Alu.mult, Alu.add)
                    u3s.append(u3)

                # one cross-partition reduce for the whole group
                s4 = stat_pool.tile([P, GROUP], f32, tag="s4")
                nc.gpsimd.partition_all_reduce(
                    s4, rs4g, channels=P, reduce_op=bass_isa.ReduceOp.add)

                for k in range(GROUP):
                    i = i0 + k
                    dcol = stat_pool.tile([P, 1], f32, tag="dcol")
                    nc.vector.tensor_scalar(dcol, s4[:, k:k + 1],
                                            bc["fb"][:, i:i + 1],
                                            bc["bb"][:, i:i + 1],
                                            Alu.mult, Alu.add)
                    # t = c*u3 + D on ACT ; out = clip(t, 0, 1) on DVE
                    tt = work_pool.tile([P, FREE], f32, tag="tt")
                    nc.scalar.activation(tt, u3s[k], Act.Identity,
                                         bias=dcol[:, 0:1],
                                         scale=bc["c"][:, i:i + 1])
                    nc.vector.tensor_scalar(og[:, k * FREE:(k + 1) * FREE],
                                            tt, 1.0, 0.0, Alu.min, Alu.max)

                dst = out[i0:i0 + GROUP].rearrange(
                    "b c h w -> b (c h w)").rearrange(
                    "b (p f) -> p b f", p=P)
                nc.scalar.dma_start(
                    dst, og.rearrange("p (b f) -> p b f", b=GROUP))

    nc.compile()
    return nc


def kernel(x: np.ndarray, sample: np.ndarray) -> np.ndarray:
    x = np.ascontiguousarray(np.asarray(x, dtype=np.float32))
    sample = np.asarray(sample)
    samp32 = sample.astype(np.int32)

    # sharding policy: within each core's 32 images, order flipud images
    # first so the bass program only needs dual predicated loads for the
    # first K slots (K = max flipud count over cores).
    orders = []
    n_flip = 0
    for c in range(N_CORES):
        sl = samp32[c * B_LOC:(c + 1) * B_LOC]
        order = np.argsort(sl != 2, kind="stable")
        orders.append(order)
        n_flip = max(n_flip, int((sl == 2).sum()))

    if n_flip not in _CACHE:
        _CACHE[n_flip] = _build_nc(n_flip)
    nc = _CACHE[n_flip]

    in_maps = []
    for c in range(N_CORES):
        o = orders[c]
        in_maps.append({
            "x": np.ascontiguousarray(x[c * B_LOC:(c + 1) * B_LOC][o]),
            "sample": np.ascontiguousarray(samp32[c * B_LOC:(c + 1) * B_LOC][o]),
        })
    res = run_bass_kernel_spmd(nc, in_maps, core_ids=list(range(N_CORES)),
                               trace=_TRACE["on"])
    if _TRACE["on"]:
        _TRACE["res"] = res
    outs = []
    for c in range(N_CORES):
        o = orders[c]
        inv = np.empty_like(o)
        inv[o] = np.arange(B_LOC)
        outs.append(res.results[c]["out"][inv])
    return np.concatenate(outs, axis=0).astype(np.float32)


# revision 14
# speedup vs baseline: 1.1113x; 1.1113x over previous
"""Trainium2 Bass kernel v6: per-image routed data augmentation (moe_routing).

For each image i, apply transform sample[i]:
  0: identity  1: fliplr  2: flipud  3: brightness(clip(1.5x,0,1))
  4: contrast(clip(1.5(x-mean)+mean,0,1))  5: solarize(x<0.5 ? x : 1-x)

Learned from HW profiles of earlier versions:
  v3 (264us): predicated-false DMAs still read their full source, and the
      flipud DRAM round-trip tripled read traffic (57.8MB read vs 19.3 needed).
  v4 (282us): SWDGE (gpsimd) casting DMAs serialize on Q7 descriptor gen.
  v5 (320us): PE matmul flipud-selection costs 256 matmuls + 256 ldweights
      (214us at mid p-state) and gpsimd tensor ops are ~2us each.

v6 keeps the lean single-pass ALU pipeline of v5 but does flipud at load
time: the channel-aligned P=96 layout (partition 32c+pc holds rows
[7pc,7pc+7) of channel c) makes the flipped image expressible as ONE
strided DMA from x ([3,32,7,224] with negative h strides), so each image
issues two mutually-exclusive predicated loads (normal / flipped) and no
other flipud machinery exists.  Phantom reads of the skipped variant cost
+19.3MB but remove all PE/PSUM work.

Per image (32 per core, pure data parallel on 8 cores):
    DMA   2 mutually-exclusive predicated loads (SP queue)
    ACT   g = Prelu_al(s*v + b), one instr, accum_out -> row sums rs
    Pool  partition_all_reduce rs -> S (broadcast column)
    DVE   D = fb*S + Dstat (tiny)
    DVE   u3 = Bp*v_wrev + g
    DVE   t = c*u3 + D ; out = clip(t, 0, 1)
    DMA   static batched store, 4 images per DMA (ACT queue)

Transform coefficient table (per-image columns):
    t=0 identity:   s=-1, al=1,  b=0,   Bp=0,  c=-1, D=0
    t=1 fliplr:     s=-1, al=0,  b=0,   Bp=-1, c=-1, D=0
    t=2 flipud:     s=-1, al=1,  b=0,   Bp=0,  c=-1, D=0   (flipped load)
    t=3 brightness: s=1.5,al=1,  b=0,   Bp=0,  c=+1, D=0      (clip does it)
    t=4 contrast:   s=1.5,al=1,  b=0,   Bp=0,  c=+1, D=-S_g/(3*PIX)
    t=5 solarize:   s=-1, al=-1, b=1/2, Bp=0,  c=-1, D=1/2
"""

import numpy as np

import concourse.bass as bass
import concourse.bacc as bacc
import concourse.mybir as mybir
import concourse.bass_isa as bass_isa
from concourse.tile import TileContext
from concourse.bass_utils import run_bass_kernel_spmd

N_CORES = 8
B = 256
B_LOC = B // N_CORES          # 32 images per core
C, H, W = 3, 224, 224
PIX = C * H * W               # 150528
P = 96                        # partitions; 32 per channel, 7 H-rows each
RPP = 7                       # H-rows per partition
FREE = RPP * W                # 1568
GROUP = 4                     # images per store DMA

f32 = mybir.dt.float32
i32 = mybir.dt.int32
Alu = mybir.AluOpType
Act = mybir.ActivationFunctionType

_CACHE = {}
# test harness hook: when _TRACE["on"], kernel() captures an NTFF profile
_TRACE = {"on": False, "res": None}


def _build_nc(n_flip: int):
    nc = bacc.Bacc()
    x = nc.declare_dram_parameter("x", [B_LOC, C, H, W], f32, isOutput=False)
    samp = nc.declare_dram_parameter("sample", [B_LOC], i32, isOutput=False)
    out = nc.declare_dram_parameter("out", [B_LOC, C, H, W], f32, isOutput=True)

    with TileContext(nc) as tc:
        with (
            tc.tile_pool(name="coef", bufs=1) as coef_pool,
            tc.tile_pool(name="data", bufs=6) as data_pool,
            tc.tile_pool(name="work", bufs=3) as work_pool,
            tc.tile_pool(name="outp", bufs=2) as out_pool,
            tc.tile_pool(name="stat", bufs=4) as stat_pool,
        ):
            # ---------- routing coefficients from sample ----------
            s_i = coef_pool.tile([1, B_LOC], i32)
            nc.sync.dma_start(s_i, samp[:].unsqueeze(0))
            s_f = coef_pool.tile([1, B_LOC], f32)
            nc.vector.tensor_copy(s_f, s_i)

            m = {}
            for k in (1, 3, 4, 5):
                mk = coef_pool.tile([1, B_LOC], f32, tag=f"mask{k}")
                nc.vector.tensor_scalar(mk, s_f, float(k), None, Alu.is_equal)
                m[k] = mk
            m34 = coef_pool.tile([1, B_LOC], f32)
            nc.vector.tensor_tensor(m34, m[3], m[4], Alu.add)

            rows = {}
            sc_row = coef_pool.tile([1, B_LOC], f32)   # s = -1 + 2.5*m34
            nc.vector.tensor_scalar(sc_row, m34, 2.5, -1.0, Alu.mult, Alu.add)
            rows["sc"] = sc_row
            t1 = coef_pool.tile([1, B_LOC], f32)       # al = 1 - m1 - 2*m5
            nc.vector.scalar_tensor_tensor(t1, m[5], 2.0, m[1], Alu.mult, Alu.add)
            al_row = coef_pool.tile([1, B_LOC], f32)
            nc.vector.tensor_scalar(al_row, t1, -1.0, 1.0, Alu.mult, Alu.add)
            rows["al"] = al_row
            bb_row = coef_pool.tile([1, B_LOC], f32)   # b = Dstat = 0.5*m5
            nc.vector.tensor_scalar(bb_row, m[5], 0.5, None, Alu.mult)
            rows["bb"] = bb_row
            bp_row = coef_pool.tile([1, B_LOC], f32)   # Bp = -m1
            nc.vector.tensor_scalar(bp_row, m[1], -1.0, None, Alu.mult)
            rows["bp"] = bp_row
            c_row = coef_pool.tile([1, B_LOC], f32)    # c = -1 + 2*m34
            nc.vector.tensor_scalar(c_row, m34, 2.0, -1.0, Alu.mult, Alu.add)
            rows["c"] = c_row
            fb_row = coef_pool.tile([1, B_LOC], f32)   # fb = -m4/(3*PIX)
            nc.vector.tensor_scalar(fb_row, m[4], -1.0 / (3.0 * PIX), None,
                                    Alu.mult)
            rows["fb"] = fb_row

            bc = {}
            for name, row in rows.items():
                t = coef_pool.tile([P, B_LOC], f32, tag=f"bc_{name}")
                nc.gpsimd.partition_broadcast(t, row)
                bc[name] = t

            # int flags for the predicated loads
            is_ud = coef_pool.tile([1, B_LOC], i32)
            nc.vector.tensor_scalar(is_ud, s_i, 2, None, Alu.is_equal)
            not_ud = coef_pool.tile([1, B_LOC], i32)
            nc.vector.tensor_scalar(not_ud, s_i, 2, None, Alu.not_equal)

            # ---------- main loop, one 4-image group at a time ----------
            # The gpsimd cross-partition reduce is batched per group: each
            # image's ACT accum lands in one column of rs4g, a single
            # partition_all_reduce handles all four, and the tail ops
            # (D column, final affine + clamp) run after it.
            # software-pipelined: group g's tail ops (which depend on the
            # gpsimd all_reduce round trip) are emitted after group g+1's
            # head ops, so the reduce latency hides behind a full group.
            n_groups = B_LOC // GROUP
            pending = None

            def emit_tail(p):
                i0p, u3sp, s4p = p
                og = out_pool.tile([P, GROUP * FREE], f32, tag="og")
                for k in range(GROUP):
                    i = i0p + k
                    dcol = stat_pool.tile([P, 1], f32, tag="dcol")
                    nc.vector.tensor_scalar(dcol, s4p[:, k:k + 1],
                                            bc["fb"][:, i:i + 1],
                                            bc["bb"][:, i:i + 1],
                                            Alu.mult, Alu.add)
                    # t = c*u3 + D ; out = clip(t, 0, 1)
                    tt = work_pool.tile([P, FREE], f32, tag="tt")
                    nc.vector.tensor_scalar(tt, u3sp[k], bc["c"][:, i:i + 1],
                                            dcol[:, 0:1], Alu.mult, Alu.add)
                    nc.vector.tensor_scalar(og[:, k * FREE:(k + 1) * FREE],
                                            tt, 1.0, 0.0, Alu.min, Alu.max)
                dst = out[i0p:i0p + GROUP].rearrange(
                    "b c h w -> b (c h w)").rearrange(
                    "b (p f) -> p b f", p=P)
                nc.scalar.dma_start(
                    dst, og.rearrange("p (b f) -> p b f", b=GROUP))

            for gi in range(n_groups):
                i0 = gi * GROUP
                rs4g = stat_pool.tile([P, GROUP], f32, tag="rs4g")
                u3s = []
                for k in range(GROUP):
                    i = i0 + k
                    # host-side sort puts all flipud images in the first
                    # n_flip slots; only those need the expensive dual
                    # predicated loads (the flipped AP walks 672 DGE
                    # descriptors whether skipped or not).
                    T = data_pool.tile([P, FREE], f32, tag="T")
                    src_n = x[i].flatten().rearrange("(p f) -> p f", p=P)
                    if i < n_flip:
                        cond_n = nc.values_load(
                            not_ud[0:1, i:i + 1],
                            engines=(mybir.EngineType.SP,),
                            min_val=0, max_val=1,
                            skip_runtime_bounds_check=True)
                        cond_u = nc.values_load(
                            is_ud[0:1, i:i + 1],
                            engines=(mybir.EngineType.SP,),
                            min_val=0, max_val=1,
                            skip_runtime_bounds_check=True)
                        nc.sync.dma_start(T, src_n, cond=cond_n,
                                          cond_hint=True)
                        src_u = x[i, :, ::-1, :].rearrange(
                            "c (p r) w -> c p r w", p=32)
                        nc.sync.dma_start(T, src_u, cond=cond_u,
                                          cond_hint=False)
                    else:
                        nc.sync.dma_start(T, src_n)

                    T3 = T.rearrange("p (r w) -> p r w", r=RPP)

                    # ACT1: g = Prelu_al(s*v + b), accum -> per-partition sums
                    g = work_pool.tile([P, FREE], f32, tag="g")
                    nc.scalar.activation(
                        g, T, Act.Prelu,
                        bias=bc["bb"][:, i:i + 1],
                        scale=bc["sc"][:, i:i + 1],
                        alpha=bc["al"][:, i:i + 1],
                        accum_out=rs4g[:, k:k + 1],
                    )

                    # u3 = Bp * v_wrev + g
                    u3 = work_pool.tile([P, FREE], f32, tag="u3", bufs=9)
                    nc.vector.scalar_tensor_tensor(
                        u3.rearrange("p (r w) -> p r w", r=RPP),
                        T3[:, :, ::-1], bc["bp"][:, i:i + 1],
                        g.rearrange("p (r w) -> p r w", r=RPP),
                        Alu.mult, Alu.add)
                    u3s.append(u3)

                # one cross-partition reduce for the whole group
                s4 = stat_pool.tile([P, GROUP], f32, tag="s4")
                nc.gpsimd.partition_all_reduce(
                    s4, rs4g, channels=P, reduce_op=bass_isa.ReduceOp.add)

                if pending is not None:
                    emit_tail(pending)
                pending = (i0, u3s, s4)

            emit_tail(pending)

    nc.compile()
    return nc


def kernel(x: np.ndarray, sample: np.ndarray) -> np.ndarray:
    x = np.ascontiguousarray(np.asarray(x, dtype=np.float32))
    sample = np.asarray(sample)
    samp32 = sample.astype(np.int32)

    # sharding policy: within each core's 32 images, order flipud images
    # first so the bass program only needs dual predicated loads for the
    # first K slots (K = max flipud count over cores).
    orders = []
    n_flip = 0
    for c in range(N_CORES):
        sl = samp32[c * B_LOC:(c + 1) * B_LOC]
        order = np.argsort(sl != 2, kind="stable")
        orders.append(order)
        n_flip = max(n_flip, int((sl == 2).sum()))

    if n_flip not in _CACHE:
        _CACHE[n_flip] = _build_nc(n_flip)
    nc = _CACHE[n_flip]

    in_maps = []
    for c in range(N_CORES):
        o = orders[c]
        in_maps.append({
            "x": np.ascontiguousarray(x[c * B_LOC:(c + 1) * B_LOC][o]),
            "sample": np.ascontiguousarray(samp32[c * B_LOC:(c + 1) * B_LOC][o]),
        })
    res = run_bass_kernel_spmd(nc, in_maps, core_ids=list(range(N_CORES)),
                               trace=_TRACE["on"])
    if _TRACE["on"]:
        _TRACE["res"] = res
    outs = []
    for c in range(N_CORES):
        o = orders[c]
        inv = np.empty_like(o)
        inv[o] = np.arange(B_LOC)
        outs.append(res.results[c]["out"][inv])
    return np.concatenate(outs, axis=0).astype(np.float32)
